# revision 1
# baseline (speedup 1.0000x reference)
"""Trainium2 Bass kernel for nn_DAGModel (gnn_message_passing).

Strategy (data-parallel over batch, 8 b's per core):
- node_vecs live in DRAM as a bf16 table `nv[token, b8, h128]` (2KB rows,
  all 8 local batch elements interleaved per token so one gather serves
  all of them).
- Parent gathers use the GPSIMD bulk `dma_gather(transpose=True)` which
  lands feature-major ([h on partitions, gather-position on free]) —
  exactly the matmul layout, no on-chip transposes on the input side.
- Nodes of each depth are reordered (host-side renumbering) by descending
  parent count so parent-slot j's gather list is a prefix — the 8-slot
  parent sum becomes a handful of regular strided adds over ~4.5/8 of the
  naive volume. The parent-sum accumulates in f32 on DVE/GPSIMD.
- The 2-layer MLP runs in f32 on the PE (h on partitions); the residual
  `pv` is injected into the second matmul's PSUM via an identity matmul.
- new vecs are PE-transposed back to row-major, cast to bf16, and DMA'd
  to the next depth's token rows; the output projection row `out[t] =
  nv[t]·Wout[t]` is fused as a DVE scalar_tensor_tensor accumulate on the
  f32 transpose output, so the final [B,N] readout costs no extra pass
  over node_vecs.
"""

import numpy as np
import ml_dtypes

BF16 = ml_dtypes.bfloat16

# Full-problem dims (hardcoded per contract).
B, H, E = 64, 128, 128
D_FULL, P_FULL, MP = 20, 1000, 8
NCORES, BL = 8, 8
LAST_RESULTS = None


# ---------------------------------------------------------------------------
# workaround: this walrus build rejects >1 sync-wait on a CTRL (Drain) inst.
def _install_tilefix():
    import concourse.tile as tile_mod
    from concourse.vector_clock import ScopedClock, VectorClock

    if getattr(tile_mod.TileContext, "_drain_split_installed", False):
        return

    def _split_drain_and_barrier(self, tick_clock, wait_clock):
        gc = tick_clock.global_clock
        ticks = list(gc)
        nz = [(i, t) for i, t in enumerate(ticks) if t > 0]
        if nz:
            for i, t in nz:
                vec = [0] * len(ticks)
                vec[i] = t
                d = self.nc.sync.drain()
                wait_clock.add_sem_waits(
                    d.ins, ScopedClock({None: VectorClock(vec)})
                )
        else:
            d = self.nc.sync.drain()
            wait_clock.add_sem_waits(d.ins, ScopedClock({None: gc}))
        self.nc.all_engine_barrier()
        assert self.sems is not None
        popped = self.nc._tile_sem_poison_stack.pop()
        assert popped is self._sem_poison
        self.nc.clear_and_free_semaphores(list(self.sems.allocated().values()))
        self.nc.all_engine_barrier()

    tile_mod.TileContext._drain_and_barrier = _split_drain_and_barrier
    tile_mod.TileContext._drain_split_installed = True


# ---------------------------------------------------------------------------
def _wrap_idx(seq):
    """int16 index layout for dma_gather: position i -> [i%16, i//16],
    replicated across the 8 groups of 16 partitions."""
    a = np.asarray(seq, np.int16)
    L = len(a)
    assert L % 16 == 0
    a16 = a.reshape(L // 16, 16).T  # [16, L/16]
    return np.ascontiguousarray(np.tile(a16, (8, 1)))  # [128, L/16]


def _prepare(inputs, D, P, CH, NCH):
    """Host-side index preprocessing. Only index tensors are transformed;
    all float compute stays on device (except the single root output row)."""
    PP = CH * NCH
    node_indices = np.asarray(inputs["node_indices"])
    parent_indices = np.asarray(inputs["parent_indices"])
    k = (parent_indices > 0).sum(-1)  # [D, P]

    remap = np.zeros(2 + D * P, np.int64)
    remap[1] = 1
    perms = []
    for d in range(D):
        perm = np.argsort(-k[d], kind="stable")
        perms.append(perm)
        remap[2 + d * P + perm] = 2 + d * PP + np.arange(P)

    # the HW dma_gather path crashes above ~900 indices per instruction;
    # bundle the per-slot gather lists into <=BCAP-index instructions.
    BCAP = 768
    chunk_meta = []  # [d][c] -> dict(bundles=[L..], groups={j: (b, off, m)})
    pidx_wrapped = {}
    neidx_wrapped = {}
    MAXB = 0
    for d in range(D):
        perm = perms[d]
        kd = k[d][perm]
        row = []
        for c in range(NCH):
            lo = c * CH
            hi = min(lo + CH, P)
            nodes = perm[lo:hi]
            kc = kd[lo:hi]
            nreal = len(nodes)
            glists = [np.pad(remap[parent_indices[d, nodes, 0]], (0, CH - nreal))]
            for j in range(1, MP):
                mj = int((kc > j).sum())
                glists.append(remap[parent_indices[d, nodes[:mj], j]] if mj else None)
            bundles = []
            groups = {}
            cur, cur_len = [], 0
            bidx = 0
            for j in range(MP):
                g = glists[j]
                glen = 0 if g is None else len(g)
                if glen == 0:
                    groups[j] = (0, 0, 0)
                    continue
                if cur_len + glen > BCAP and cur_len > 0:
                    bundles.append((bidx, cur, cur_len))
                    bidx += 1
                    cur, cur_len = [], 0
                groups[j] = (bidx, cur_len, glen)
                cur.append(g)
                cur_len += glen
            if cur_len:
                bundles.append((bidx, cur, cur_len))
            blens = []
            for bi, parts, blen in bundles:
                idx = np.concatenate(parts)
                L = (len(idx) + 127) // 128 * 128
                idx = np.pad(idx, (0, L - len(idx)))
                pidx_wrapped[(d, c, bi)] = _wrap_idx(idx)
                blens.append(L)
            MAXB = max(MAXB, len(blens))
            ne = np.pad(node_indices[d][nodes], (0, CH - nreal))
            neidx_wrapped[(d, c)] = _wrap_idx(ne)
            row.append({"bundles": blens, "groups": groups})
        chunk_meta.append(row)

    LBMAX = max(
        (L for meta_row in chunk_meta for md in meta_row for L in md["bundles"]),
        default=128,
    )
    pidx_np = np.zeros((D, NCH, MAXB, 128, LBMAX // 16), np.int16)
    neidx_np = np.zeros((D, NCH, 128, CH // 16), np.int16)
    for d in range(D):
        for c in range(NCH):
            for bi, L in enumerate(chunk_meta[d][c]["bundles"]):
                w = pidx_wrapped[(d, c, bi)]
                pidx_np[d, c, bi, :, : w.shape[1]] = w
            neidx_np[d, c] = neidx_wrapped[(d, c)]

    W1 = np.asarray(inputs["W1"], np.float32)
    W2 = np.asarray(inputs["W2"], np.float32)
    Wout = np.asarray(inputs["Wout"], np.float32)
    bout = np.asarray(inputs["bout"], np.float32)
    emb = np.asarray(inputs["emb_table"], np.float32)

    wout_perm = np.zeros((D, PP, H), np.float32)
    wo = Wout[1:].reshape(D, P, H)
    for d in range(D):
        wout_perm[d, :P] = wo[d][perms[d]]

    prep = {
        "meta": chunk_meta,
        "perms": perms,
        "pidx": pidx_np,
        "neidx": neidx_np,
        "embt": np.ascontiguousarray(emb.astype(BF16)),
        "w1at": np.ascontiguousarray(W1[:, :H].T.astype(np.float32)),
        "w1bt": np.ascontiguousarray(W1[:, H:].T.astype(np.float32)),
        "w2t": np.ascontiguousarray(W2.T.astype(np.float32)),
        "ident": np.eye(128, dtype=np.float32),
        "b1": np.asarray(inputs["b1"], np.float32).reshape(128, 1),
        "b2": np.asarray(inputs["b2"], np.float32).reshape(128, 1),
        "woutp": wout_perm,
    }
    return prep


def _build(prep, D, P, CH, NCH):
    """Trace the Bass/Tile kernel. Returns a finalized Bacc."""
    import os
    STAGE = int(os.environ.get("KSTAGE", "9"))
    _install_tilefix()
    from contextlib import ExitStack

    import concourse.bacc as bacc
    import concourse.mybir as mybir
    from concourse.tile import TileContext

    PP = CH * NCH
    TOK = 2 + D * PP
    ROW = BL * H  # nv row elems (bf16)
    KB = CH // 128  # 128-blocks per chunk
    f32 = mybir.dt.float32
    bf16 = mybir.dt.bfloat16
    i16 = mybir.dt.int16
    AF = mybir.ActivationFunctionType
    ALU = mybir.AluOpType

    nc = bacc.Bacc("TRN2", target_bir_lowering=False, debug=False)

    nv = nc.dram_tensor("nv", [TOK, ROW], bf16, kind="Internal")
    nvinit = nc.dram_tensor("nvinit", [2, ROW], bf16, kind="ExternalInput")
    emb_rows = 2 + D * P  # emb_table rows = N+1
    embt = nc.dram_tensor("embt", [emb_rows, H], bf16, kind="ExternalInput")
    pidx_in = nc.dram_tensor(
        "pidx", list(prep["pidx"].shape), i16, kind="ExternalInput"
    )
    neidx_in = nc.dram_tensor(
        "neidx", list(prep["neidx"].shape), i16, kind="ExternalInput"
    )
    w1at_in = nc.dram_tensor("w1at", [128, 128], f32, kind="ExternalInput")
    w1bt_in = nc.dram_tensor("w1bt", [128, 128], f32, kind="ExternalInput")
    w2t_in = nc.dram_tensor("w2t", [128, 128], f32, kind="ExternalInput")
    ident_in = nc.dram_tensor("ident", [128, 128], f32, kind="ExternalInput")
    b1_in = nc.dram_tensor("b1c", [128, 1], f32, kind="ExternalInput")
    b2_in = nc.dram_tensor("b2c", [128, 1], f32, kind="ExternalInput")
    woutp_in = nc.dram_tensor("woutp", [D, PP, H], f32, kind="ExternalInput")
    outd = nc.dram_tensor("outd", [D, NCH, 128, KB, BL], f32, kind="ExternalOutput")

    meta = prep["meta"]

    with TileContext(nc) as tc, ExitStack() as ctx:
        const = ctx.enter_context(tc.tile_pool(name="const", bufs=1))
        pidx_pool = ctx.enter_context(tc.tile_pool(name="pidx", bufs=2))
        neidx_pool = ctx.enter_context(tc.tile_pool(name="neidx", bufs=2))
        stag_pool = ctx.enter_context(tc.tile_pool(name="stag", bufs=2))
        pv_pool = ctx.enter_context(tc.tile_pool(name="pv", bufs=2))
        ne_pool = ctx.enter_context(tc.tile_pool(name="ne", bufs=2))
        nef_pool = ctx.enter_context(tc.tile_pool(name="nef", bufs=2))
        h1_pool = ctx.enter_context(tc.tile_pool(name="h1", bufs=3))
        nvn_pool = ctx.enter_context(tc.tile_pool(name="nvn", bufs=6))
        nvrm_pool = ctx.enter_context(tc.tile_pool(name="nvrm", bufs=3))
        wout_pool = ctx.enter_context(tc.tile_pool(name="wout", bufs=3))
        outsb_pool = ctx.enter_context(tc.tile_pool(name="outsb", bufs=2))
        scr_pool = ctx.enter_context(tc.tile_pool(name="scr", bufs=2))
        psum_mm = ctx.enter_context(tc.tile_pool(name="psmm", bufs=2, space="PSUM"))
        psum_mm2 = ctx.enter_context(tc.tile_pool(name="psm2", bufs=2, space="PSUM"))
        psum_tp = ctx.enter_context(tc.tile_pool(name="pstp", bufs=4, space="PSUM"))

        w1at = const.tile([128, 128], f32)
        nc.sync.dma_start(out=w1at[:], in_=w1at_in[:, :])
        w1bt = const.tile([128, 128], f32)
        nc.sync.dma_start(out=w1bt[:], in_=w1bt_in[:, :])
        w2t = const.tile([128, 128], f32)
        nc.sync.dma_start(out=w2t[:], in_=w2t_in[:, :])
        ident = const.tile([128, 128], f32)
        nc.sync.dma_start(out=ident[:], in_=ident_in[:, :])
        b1 = const.tile([128, 1], f32)
        nc.sync.dma_start(out=b1[:], in_=b1_in[:, :])
        b2 = const.tile([128, 1], f32)
        nc.sync.dma_start(out=b2[:], in_=b2_in[:, :])

        # init nv rows 0..1 (zero pad row + root = per-b embedding)
        nvi = const.tile([2, ROW], bf16)
        nc.sync.dma_start(out=nvi[:], in_=nvinit[:, :])
        nc.sync.dma_start(out=nv[0:2, :], in_=nvi[:])

        for d in range(D):
            for c in range(NCH):
                md = meta[d][c]
                blens, groups = md["bundles"], md["groups"]

                stags = []
                for bi, L in enumerate(blens):
                    pidx_sb = pidx_pool.tile([128, L // 16], i16, tag=f"pidx{bi}")
                    nc.sync.dma_start(
                        out=pidx_sb[:], in_=pidx_in[d, c, bi, :, : L // 16]
                    )
                    stag = stag_pool.tile([128, BL, L], bf16, tag=f"stag{bi}")
                    # bound the source AP to rows of prior depths: parents
                    # always live below this depth's slab, and the tighter
                    # address range keeps Tile from serializing this gather
                    # against the current depth's write-backs.
                    nc.gpsimd.dma_gather(
                        stag[:], nv[0 : 2 + d * PP, :], pidx_sb[:],
                        num_idxs=L, num_idxs_reg=L,
                        elem_size=ROW, transpose=True,
                    )
                    stags.append(stag)

                def gseg(j):
                    bi, go, glen = groups[j]
                    return stags[bi][:, :, go : go + glen], glen

                nidx_sb = neidx_pool.tile([128, CH // 16], i16)
                nc.sync.dma_start(out=nidx_sb[:], in_=neidx_in[d, c, :, :])
                ne = ne_pool.tile([128, 1, CH], bf16)
                nc.gpsimd.dma_gather(
                    ne[:], embt[:, :], nidx_sb[:],
                    num_idxs=CH, num_idxs_reg=CH,
                    elem_size=H, transpose=True,
                )
                nef = nef_pool.tile([128, 1, CH], f32)
                nc.scalar.copy(nef[:], ne[:])
                if STAGE < 2:
                    continue

                # ---- parent-slot reduction -> pv f32 [128, BL, CH]
                pv = pv_pool.tile([128, BL, CH], f32)
                g0, _ = gseg(0)
                g1, m1 = gseg(1)
                if m1 > 0:
                    nc.vector.tensor_add(pv[:, :, :m1], g0[:, :, :m1], g1)
                if m1 < CH:
                    nc.vector.tensor_copy(out=pv[:, :, m1:CH], in_=g0[:, :, m1:CH])
                for j in range(2, MP):
                    gj, mj = gseg(j)
                    if mj == 0:
                        continue
                    eng = nc.vector if j % 2 == 0 else nc.gpsimd
                    eng.tensor_add(pv[:, :, :mj], pv[:, :, :mj], gj)

                if STAGE < 3:
                    continue
                # ---- MLP (f32) over col pairs (2 b's x CH = 512 cols)
                nvns = []
                for bp in range(BL // 2):
                    rhs_pv = pv[:, 2 * bp : 2 * bp + 2, :]
                    h1p = psum_mm.tile([128, 2, CH], f32, tag="h1p")
                    nc.tensor.matmul(
                        h1p[:], lhsT=w1at[:], rhs=rhs_pv, start=True, stop=False
                    )
                    nc.tensor.matmul(
                        h1p[:],
                        lhsT=w1bt[:],
                        rhs=nef[:].to_broadcast([128, 2, CH]),
                        start=False,
                        stop=True,
                    )
                    h1 = h1_pool.tile([128, 2, CH], f32)
                    nc.scalar.activation(h1[:], h1p[:], AF.Relu, bias=b1[:])
                    h2p = psum_mm2.tile([128, 2, CH], f32, tag="h2p")
                    nc.tensor.matmul(
                        h2p[:], lhsT=w2t[:], rhs=h1[:], start=True, stop=False
                    )
                    nc.tensor.matmul(
                        h2p[:], lhsT=ident[:], rhs=rhs_pv, start=False, stop=True
                    )
                    nvn = nvn_pool.tile([128, 2, CH], f32)
                    nc.scalar.activation(nvn[:], h2p[:], AF.Identity, bias=b2[:])
                    nvns.append(nvn)

                if STAGE < 4:
                    continue
                # ---- transpose back, cast, write-back, fused out-projection
                outsb = outsb_pool.tile([128, KB * BL], f32)
                for kb in range(KB):
                    kbg = c * KB + kb
                    wout_sb = wout_pool.tile([128, 128], f32)
                    nc.sync.dma_start(
                        out=wout_sb[:],
                        in_=woutp_in[d, kbg * 128 : (kbg + 1) * 128, :],
                    )
                    nvrm = nvrm_pool.tile([128, BL, 128], bf16)
                    for half in range(2):
                        tp = psum_tp.tile([128, 4, 128], f32, tag="tp")
                        for bq in range(4):
                            b = half * 4 + bq
                            nc.tensor.transpose(
                                tp[:, bq, :],
                                nvns[b // 2][:, b % 2, kb * 128 : (kb + 1) * 128],
                                ident[:],
                            )
                        nc.scalar.copy(
                            out=nvrm[:, half * 4 : half * 4 + 4, :], in_=tp[:]
                        )
                        for bq in range(4):
                            if STAGE < 5:
                                continue
                            b = half * 4 + bq
                            scr = scr_pool.tile([128, 128], f32)
                            nc.vector.scalar_tensor_tensor(
                                out=scr[:],
                                in0=tp[:, bq, :],
                                scalar=1.0,
                                in1=wout_sb[:],
                                op0=ALU.mult,
                                op1=ALU.mult,
                                accum_out=outsb[:, kb * BL + b : kb * BL + b + 1],
                            )
                    tokbase = 2 + d * PP + c * CH + kb * 128
                    nc.sync.dma_start(
                        out=nv[tokbase : tokbase + 128, :],
                        in_=nvrm[:].rearrange("p b h -> p (b h)"),
                    )
                if STAGE >= 5:
                    nc.sync.dma_start(
                        out=outd[d, c, :, :, :],
                        in_=outsb[:].rearrange("p (k b) -> p k b", k=KB),
                    )

    nc.finalize()
    return nc


def _run_cores(nc, prep, embedding, n_cores):
    from concourse import bass_utils

    in_maps = []
    base = {
        "embt": prep["embt"],
        "pidx": prep["pidx"],
        "neidx": prep["neidx"],
        "w1at": prep["w1at"],
        "w1bt": prep["w1bt"],
        "w2t": prep["w2t"],
        "ident": prep["ident"],
        "b1c": prep["b1"],
        "b2c": prep["b2"],
        "woutp": prep["woutp"],
    }
    for core in range(n_cores):
        eb = embedding[core * BL : (core + 1) * BL]  # [BL, H]
        nvinit = np.zeros((2, BL * H), np.float32)
        nvinit[1] = eb.reshape(-1)
        m = dict(base)
        m["nvinit"] = np.ascontiguousarray(nvinit.astype(BF16))
        in_maps.append(m)
    res = bass_utils.run_bass_kernel_spmd(
        nc, in_maps, core_ids=list(range(n_cores))
    )
    global LAST_RESULTS
    LAST_RESULTS = res
    return res


def _assemble(results, prep, inputs, D, P, CH, NCH, n_cores):
    PP = CH * NCH
    KB = CH // 128
    embedding = np.asarray(inputs["embedding"], np.float32)
    Wout = np.asarray(inputs["Wout"], np.float32)
    bout = np.asarray(inputs["bout"], np.float32)
    NTOT = 1 + D * P

    out = np.empty((embedding.shape[0], NTOT), np.float32)
    out[:, 0] = embedding @ Wout[0] + bout[0]
    for core in range(n_cores):
        v = results[core]["outd"]  # [D, NCH, 128, KB, BL]
        v = v.transpose(0, 1, 3, 2, 4).reshape(D, PP, BL)  # s = c*CH + kb*128 + n
        for d in range(D):
            perm = prep["perms"][d]
            cols = 1 + d * P + perm  # output column for sorted position s
            out[core * BL : (core + 1) * BL, cols] = v[d, :P].T
    out[:, 1:] += bout[None, 1:]
    return out


def kernel(**inputs):
    D, P, CH, NCH = D_FULL, P_FULL, 256, 4
    prep = _prepare(inputs, D, P, CH, NCH)
    nc = _build(prep, D, P, CH, NCH)
    res = _run_cores(nc, prep, np.asarray(inputs["embedding"], np.float32), NCORES)
    return _assemble(res.results, prep, inputs, D, P, CH, NCH, NCORES)



# revision 2
# speedup vs baseline: 2.9007x; 2.9007x over previous
"""Trainium2 Bass kernel for nn_DAGModel (gnn_message_passing).

Strategy (data-parallel over batch, 8 b's per core):
- node_vecs live in DRAM as a bf16 table `nv[token, b8, h128]` (2KB rows,
  all 8 local batch elements interleaved per token so one gather serves
  all of them).
- Parent gathers use the GPSIMD bulk `dma_gather(transpose=True)` which
  lands feature-major ([h on partitions, gather-position on free]) —
  exactly the matmul layout, no on-chip transposes on the input side.
- Nodes of each depth are reordered (host-side renumbering) by descending
  parent count so parent-slot j's gather list is a prefix — the 8-slot
  parent sum becomes a handful of regular strided adds over ~4.5/8 of the
  naive volume. The parent-sum accumulates in bf16 on DVE.
- Depth 0 has only the root as a possible parent, so pv = count * root
  is computed on-chip with no gathers at all.
- Node embeddings are statically known (node_indices is an input), so the
  permuted embedding tiles are prepared host-side feature-major and
  streamed in — no embedding gathers.
- The 2-layer MLP runs in bf16 on the PE (h on partitions); the residual
  `pv` is injected into the second matmul's PSUM via an identity matmul.
- new vecs are PE-transposed (bf16, single pass) back to row-major and
  DMA'd to the next depth's token rows; the output projection
  out[t] = nv[t]·Wout[t] is a DVE multiply + reduce over the row-major
  tile, so the final [B,N] readout costs no extra pass over node_vecs.
- Gather source APs are bounded to the max referenced token so Tile can
  overlap next-depth gathers with this depth's trailing write-backs.
"""

import numpy as np
import ml_dtypes

BF16 = ml_dtypes.bfloat16

# Full-problem dims (hardcoded per contract).
B, H, E = 64, 128, 128
D_FULL, P_FULL, MP = 20, 1000, 8
NCORES, BL = 8, 8
LAST_RESULTS = None


# ---------------------------------------------------------------------------
# workaround: this walrus build rejects >1 sync-wait on a CTRL (Drain) inst.
def _install_tilefix():
    import concourse.tile as tile_mod
    from concourse.vector_clock import ScopedClock, VectorClock

    if getattr(tile_mod.TileContext, "_drain_split_installed", False):
        return

    def _split_drain_and_barrier(self, tick_clock, wait_clock):
        gc = tick_clock.global_clock
        ticks = list(gc)
        nz = [(i, t) for i, t in enumerate(ticks) if t > 0]
        if nz:
            for i, t in nz:
                vec = [0] * len(ticks)
                vec[i] = t
                d = self.nc.sync.drain()
                wait_clock.add_sem_waits(
                    d.ins, ScopedClock({None: VectorClock(vec)})
                )
        else:
            d = self.nc.sync.drain()
            wait_clock.add_sem_waits(d.ins, ScopedClock({None: gc}))
        self.nc.all_engine_barrier()
        assert self.sems is not None
        popped = self.nc._tile_sem_poison_stack.pop()
        assert popped is self._sem_poison
        self.nc.clear_and_free_semaphores(list(self.sems.allocated().values()))
        self.nc.all_engine_barrier()

    tile_mod.TileContext._drain_and_barrier = _split_drain_and_barrier
    tile_mod.TileContext._drain_split_installed = True


# ---------------------------------------------------------------------------
def _wrap_idx(seq):
    """int16 index layout for dma_gather: position i -> [i%16, i//16],
    replicated across the 8 groups of 16 partitions."""
    a = np.asarray(seq, np.int16)
    L = len(a)
    assert L % 16 == 0
    a16 = a.reshape(L // 16, 16).T  # [16, L/16]
    return np.ascontiguousarray(np.tile(a16, (8, 1)))  # [128, L/16]


def _prepare(inputs, D, P, CH, NCH):
    """Host-side index preprocessing. Only index tensors and statically
    known embedding/weight reorders are transformed on the host; all
    batch-dependent float compute stays on device."""
    PP = CH * NCH
    node_indices = np.asarray(inputs["node_indices"])
    parent_indices = np.asarray(inputs["parent_indices"])
    k = (parent_indices > 0).sum(-1)  # [D, P]

    remap = np.zeros(2 + D * P, np.int64)
    remap[1] = 1
    perms = []
    for d in range(D):
        perm = np.argsort(-k[d], kind="stable")
        perms.append(perm)
        remap[2 + d * P + perm] = 2 + d * PP + np.arange(P)

    # the HW dma_gather path crashes above ~900 indices per instruction;
    # bundle the per-slot gather lists into <=BCAP-index instructions.
    BCAP = 768
    chunk_meta = []  # [d][c] -> dict(bundles=[(L, maxref)..], groups={j: (b, off, m)})
    pidx_wrapped = {}
    MAXB = 0
    for d in range(D):
        perm = perms[d]
        kd = k[d][perm]
        row = []
        for c in range(NCH):
            lo = c * CH
            hi = min(lo + CH, P)
            nodes = perm[lo:hi]
            kc = kd[lo:hi]
            nreal = len(nodes)
            if d == 0:
                # depth 0: every parent is the root; pv computed on-chip.
                row.append({"bundles": [], "groups": {}})
                continue
            glists = [np.pad(remap[parent_indices[d, nodes, 0]], (0, CH - nreal))]
            for j in range(1, MP):
                mj = int((kc > j).sum())
                glists.append(remap[parent_indices[d, nodes[:mj], j]] if mj else None)
            bundles = []
            groups = {}
            cur, cur_len = [], 0
            bidx = 0
            for j in range(MP):
                g = glists[j]
                glen = 0 if g is None else len(g)
                if glen == 0:
                    groups[j] = (0, 0, 0)
                    continue
                if cur_len + glen > BCAP and cur_len > 0:
                    bundles.append((bidx, cur, cur_len))
                    bidx += 1
                    cur, cur_len = [], 0
                groups[j] = (bidx, cur_len, glen)
                cur.append(g)
                cur_len += glen
            if cur_len:
                bundles.append((bidx, cur, cur_len))
            blens = []
            for bi, parts, blen in bundles:
                idx = np.concatenate(parts)
                L = (len(idx) + 127) // 128 * 128
                maxref = int(idx.max())
                idx = np.pad(idx, (0, L - len(idx)))
                pidx_wrapped[(d, c, bi)] = _wrap_idx(idx)
                blens.append((L, maxref))
            MAXB = max(MAXB, len(blens))
            row.append({"bundles": blens, "groups": groups})
        chunk_meta.append(row)

    LBMAX = max(
        (L for meta_row in chunk_meta for md in meta_row for (L, _) in md["bundles"]),
        default=128,
    )
    pidx_np = np.zeros((D, NCH, MAXB, 128, LBMAX // 16), np.int16)
    for d in range(D):
        for c in range(NCH):
            for bi, (L, _) in enumerate(chunk_meta[d][c]["bundles"]):
                w = pidx_wrapped[(d, c, bi)]
                pidx_np[d, c, bi, :, : w.shape[1]] = w

    W1 = np.asarray(inputs["W1"], np.float32)
    W2 = np.asarray(inputs["W2"], np.float32)
    Wout = np.asarray(inputs["Wout"], np.float32)
    emb = np.asarray(inputs["emb_table"], np.float32)

    # permuted node-embedding tiles, feature-major [D, NCH, E, CH] (bf16)
    neT = np.zeros((D, NCH, H, CH), BF16)
    for d in range(D):
        for c in range(NCH):
            lo = c * CH
            hi = min(lo + CH, P)
            nodes = perms[d][lo:hi]
            ne = emb[node_indices[d][nodes]]  # [nreal, E] f32
            neT[d, c, :, : hi - lo] = ne.T.astype(BF16)

    wout_perm = np.zeros((D, PP, H), BF16)
    wo = Wout[1:].reshape(D, P, H)
    for d in range(D):
        wout_perm[d, :P] = wo[d][perms[d]].astype(BF16)

    # depth-0 parent counts in permuted order, replicated across partitions
    kcnt = np.zeros((NCH, 128, CH), BF16)
    k0 = k[0][perms[0]].astype(np.float32)  # [P]
    k0 = np.pad(k0, (0, PP - P))
    for c in range(NCH):
        kcnt[c] = np.tile(k0[c * CH : (c + 1) * CH].astype(BF16), (128, 1))

    prep = {
        "meta": chunk_meta,
        "perms": perms,
        "pidx": pidx_np,
        "neT": neT,
        "w1at": np.ascontiguousarray(W1[:, :H].T.astype(BF16)),
        "w1bt": np.ascontiguousarray(W1[:, H:].T.astype(BF16)),
        "w2t": np.ascontiguousarray(W2.T.astype(BF16)),
        "ident": np.eye(128, dtype=BF16),
        "b1": np.asarray(inputs["b1"], np.float32).reshape(128, 1),
        "b2": np.asarray(inputs["b2"], np.float32).reshape(128, 1),
        "woutp": wout_perm,
        "kcnt": kcnt,
    }
    return prep


def _build(prep, D, P, CH, NCH):
    """Trace the Bass/Tile kernel. Returns a finalized Bacc."""
    _install_tilefix()
    from contextlib import ExitStack

    import concourse.bacc as bacc
    import concourse.mybir as mybir
    from concourse.tile import TileContext

    PP = CH * NCH
    TOK = 2 + D * PP
    ROW = BL * H  # nv row elems (bf16)
    KB = CH // 128  # 128-blocks per chunk
    f32 = mybir.dt.float32
    bf16 = mybir.dt.bfloat16
    i16 = mybir.dt.int16
    AF = mybir.ActivationFunctionType
    ALU = mybir.AluOpType
    AX = mybir.AxisListType

    nc = bacc.Bacc("TRN2", target_bir_lowering=False, debug=False)

    nv = nc.dram_tensor("nv", [TOK, ROW], bf16, kind="Internal")
    nvinit = nc.dram_tensor("nvinit", [2, ROW], bf16, kind="ExternalInput")
    rootT_in = nc.dram_tensor("rootT", [128, BL], bf16, kind="ExternalInput")
    neT_in = nc.dram_tensor(
        "neT", list(prep["neT"].shape), bf16, kind="ExternalInput"
    )
    pidx_in = nc.dram_tensor(
        "pidx", list(prep["pidx"].shape), i16, kind="ExternalInput"
    )
    w1at_in = nc.dram_tensor("w1at", [128, 128], bf16, kind="ExternalInput")
    w1bt_in = nc.dram_tensor("w1bt", [128, 128], bf16, kind="ExternalInput")
    w2t_in = nc.dram_tensor("w2t", [128, 128], bf16, kind="ExternalInput")
    ident_in = nc.dram_tensor("ident", [128, 128], bf16, kind="ExternalInput")
    b1_in = nc.dram_tensor("b1c", [128, 1], f32, kind="ExternalInput")
    b2_in = nc.dram_tensor("b2c", [128, 1], f32, kind="ExternalInput")
    woutp_in = nc.dram_tensor("woutp", [D, PP, H], bf16, kind="ExternalInput")
    kcnt_in = nc.dram_tensor("kcnt", [NCH, 128, CH], bf16, kind="ExternalInput")
    outd = nc.dram_tensor("outd", [D, NCH, 128, KB, BL], f32, kind="ExternalOutput")

    meta = prep["meta"]

    with TileContext(nc) as tc, ExitStack() as ctx:
        const = ctx.enter_context(tc.tile_pool(name="const", bufs=1))
        pidx_pool = ctx.enter_context(tc.tile_pool(name="pidx", bufs=4))
        stag_pool = ctx.enter_context(tc.tile_pool(name="stag", bufs=4))
        pv_pool = ctx.enter_context(tc.tile_pool(name="pv", bufs=2))
        ne_pool = ctx.enter_context(tc.tile_pool(name="ne", bufs=2))
        h1_pool = ctx.enter_context(tc.tile_pool(name="h1", bufs=3))
        nvn_pool = ctx.enter_context(tc.tile_pool(name="nvn", bufs=6))
        nvrm_pool = ctx.enter_context(tc.tile_pool(name="nvrm", bufs=3))
        wout_pool = ctx.enter_context(tc.tile_pool(name="wout", bufs=3))
        outsb_pool = ctx.enter_context(tc.tile_pool(name="outsb", bufs=2))
        tmp_pool = ctx.enter_context(tc.tile_pool(name="tmp", bufs=2))
        psum_mm = ctx.enter_context(tc.tile_pool(name="psmm", bufs=2, space="PSUM"))
        psum_mm2 = ctx.enter_context(tc.tile_pool(name="psm2", bufs=2, space="PSUM"))
        psum_tp = ctx.enter_context(tc.tile_pool(name="pstp", bufs=4, space="PSUM"))

        w1at = const.tile([128, 128], bf16)
        nc.sync.dma_start(out=w1at[:], in_=w1at_in[:, :])
        w1bt = const.tile([128, 128], bf16)
        nc.sync.dma_start(out=w1bt[:], in_=w1bt_in[:, :])
        w2t = const.tile([128, 128], bf16)
        nc.sync.dma_start(out=w2t[:], in_=w2t_in[:, :])
        ident = const.tile([128, 128], bf16)
        nc.sync.dma_start(out=ident[:], in_=ident_in[:, :])
        b1 = const.tile([128, 1], f32)
        nc.sync.dma_start(out=b1[:], in_=b1_in[:, :])
        b2 = const.tile([128, 1], f32)
        nc.sync.dma_start(out=b2[:], in_=b2_in[:, :])
        rootT = const.tile([128, BL], bf16)
        nc.sync.dma_start(out=rootT[:], in_=rootT_in[:, :])

        # init nv rows 0..1 (zero pad row + root = per-b embedding)
        nvi = const.tile([2, ROW], bf16)
        nc.sync.dma_start(out=nvi[:], in_=nvinit[:, :])
        nc.sync.dma_start(out=nv[0:2, :], in_=nvi[:])

        for d in range(D):
            for c in range(NCH):
                md = meta[d][c]
                blens, groups = md["bundles"], md["groups"]

                # ---- parent vec gathers + slot reduction -> pv bf16
                pv = pv_pool.tile([128, BL, CH], bf16)
                if d == 0:
                    kc = ne_pool.tile([128, 1, CH], bf16, tag="kcnt")
                    nc.sync.dma_start(out=kc[:], in_=kcnt_in[c, :, :].unsqueeze(1))
                    nc.vector.tensor_tensor(
                        out=pv[:],
                        in0=rootT[:].unsqueeze(2).to_broadcast([128, BL, CH]),
                        in1=kc[:].to_broadcast([128, BL, CH]),
                        op=ALU.mult,
                    )
                else:
                    stags = []
                    for bi, (L, maxref) in enumerate(blens):
                        pidx_sb = pidx_pool.tile([128, L // 16], i16, tag=f"pidx{bi}")
                        nc.sync.dma_start(
                            out=pidx_sb[:], in_=pidx_in[d, c, bi, :, : L // 16]
                        )
                        stag = stag_pool.tile([128, BL, L], bf16, tag=f"stag{bi}")
                        # bound the source AP to the rows this bundle can
                        # reference: the tighter range lets Tile overlap this
                        # gather with later rows' write-backs.
                        nc.gpsimd.dma_gather(
                            stag[:], nv[0 : maxref + 1, :], pidx_sb[:],
                            num_idxs=L, num_idxs_reg=L,
                            elem_size=ROW, transpose=True,
                        )
                        stags.append(stag)

                    def gseg(j):
                        bi, go, glen = groups[j]
                        return stags[bi][:, :, go : go + glen], glen

                    g0, _ = gseg(0)
                    g1, m1 = gseg(1)
                    if m1 > 0:
                        nc.vector.tensor_add(pv[:, :, :m1], g0[:, :, :m1], g1)
                    if m1 < CH:
                        nc.vector.tensor_copy(out=pv[:, :, m1:CH], in_=g0[:, :, m1:CH])
                    for j in range(2, MP):
                        gj, mj = gseg(j)
                        if mj == 0:
                            continue
                        nc.vector.tensor_add(pv[:, :, :mj], pv[:, :, :mj], gj)

                ne = ne_pool.tile([128, 1, CH], bf16)
                nc.sync.dma_start(out=ne[:], in_=neT_in[d, c, :, :].unsqueeze(1))

                # ---- MLP (bf16) over col pairs (2 b's x CH = 512 cols)
                nvns = []
                for bp in range(BL // 2):
                    rhs_pv = pv[:, 2 * bp : 2 * bp + 2, :]
                    h1p = psum_mm.tile([128, 2, CH], f32, tag="h1p")
                    nc.tensor.matmul(
                        h1p[:], lhsT=w1at[:], rhs=rhs_pv, start=True, stop=False
                    )
                    nc.tensor.matmul(
                        h1p[:],
                        lhsT=w1bt[:],
                        rhs=ne[:].to_broadcast([128, 2, CH]),
                        start=False,
                        stop=True,
                    )
                    h1 = h1_pool.tile([128, 2, CH], bf16)
                    nc.scalar.activation(h1[:], h1p[:], AF.Relu, bias=b1[:])
                    h2p = psum_mm2.tile([128, 2, CH], f32, tag="h2p")
                    nc.tensor.matmul(
                        h2p[:], lhsT=w2t[:], rhs=h1[:], start=True, stop=False
                    )
                    nc.tensor.matmul(
                        h2p[:], lhsT=ident[:], rhs=rhs_pv, start=False, stop=True
                    )
                    nvn = nvn_pool.tile([128, 2, CH], bf16)
                    if bp < 2:
                        nc.scalar.activation(nvn[:], h2p[:], AF.Identity, bias=b2[:])
                    else:
                        nc.vector.tensor_scalar(
                            out=nvn[:], in0=h2p[:], scalar1=b2[:], scalar2=None,
                            op0=ALU.add,
                        )
                    nvns.append(nvn)

                # ---- transpose back (bf16), write-back, fused out-projection
                outsb = outsb_pool.tile([128, KB * BL], f32)
                for kb in range(KB):
                    kbg = c * KB + kb
                    wout_sb = wout_pool.tile([128, 128], bf16)
                    nc.sync.dma_start(
                        out=wout_sb[:],
                        in_=woutp_in[d, kbg * 128 : (kbg + 1) * 128, :],
                    )
                    nvrm = nvrm_pool.tile([128, BL, 128], bf16)
                    for half in range(2):
                        tp = psum_tp.tile([128, 4, 128], bf16, tag="tp")
                        for bq in range(4):
                            b = half * 4 + bq
                            nc.tensor.transpose(
                                tp[:, bq, :],
                                nvns[b // 2][:, b % 2, kb * 128 : (kb + 1) * 128],
                                ident[:],
                            )
                        nc.scalar.copy(
                            out=nvrm[:, half * 4 : half * 4 + 4, :], in_=tp[:]
                        )
                    # out-projection: out[t, b] = sum_h nvrm[t,b,h]*wout[t,h]
                    tmp = tmp_pool.tile([128, BL, 128], bf16)
                    nc.vector.tensor_tensor(
                        out=tmp[:],
                        in0=nvrm[:],
                        in1=wout_sb[:].unsqueeze(1).to_broadcast([128, BL, 128]),
                        op=ALU.mult,
                    )
                    nc.vector.tensor_reduce(
                        out=outsb[:, kb * BL : (kb + 1) * BL],
                        in_=tmp[:],
                        axis=AX.X,
                        op=ALU.add,
                    )
                    tokbase = 2 + d * PP + c * CH + kb * 128
                    nc.sync.dma_start(
                        out=nv[tokbase : tokbase + 128, :],
                        in_=nvrm[:].rearrange("p b h -> p (b h)"),
                    )
                nc.sync.dma_start(
                    out=outd[d, c, :, :, :],
                    in_=outsb[:].rearrange("p (k b) -> p k b", k=KB),
                )

    nc.finalize()
    return nc


def _run_cores(nc, prep, embedding, n_cores):
    from concourse import bass_utils

    in_maps = []
    base = {
        "neT": prep["neT"],
        "pidx": prep["pidx"],
        "w1at": prep["w1at"],
        "w1bt": prep["w1bt"],
        "w2t": prep["w2t"],
        "ident": prep["ident"],
        "b1c": prep["b1"],
        "b2c": prep["b2"],
        "woutp": prep["woutp"],
        "kcnt": prep["kcnt"],
    }
    for core in range(n_cores):
        eb = embedding[core * BL : (core + 1) * BL]  # [BL, H]
        nvinit = np.zeros((2, BL * H), np.float32)
        nvinit[1] = eb.reshape(-1)
        m = dict(base)
        m["nvinit"] = np.ascontiguousarray(nvinit.astype(BF16))
        m["rootT"] = np.ascontiguousarray(eb.T.astype(BF16))
        in_maps.append(m)
    res = bass_utils.run_bass_kernel_spmd(
        nc, in_maps, core_ids=list(range(n_cores))
    )
    global LAST_RESULTS
    LAST_RESULTS = res
    return res


def _assemble(results, prep, inputs, D, P, CH, NCH, n_cores):
    PP = CH * NCH
    KB = CH // 128
    embedding = np.asarray(inputs["embedding"], np.float32)
    Wout = np.asarray(inputs["Wout"], np.float32)
    bout = np.asarray(inputs["bout"], np.float32)
    NTOT = 1 + D * P

    out = np.empty((embedding.shape[0], NTOT), np.float32)
    out[:, 0] = embedding @ Wout[0] + bout[0]
    for core in range(n_cores):
        v = results[core]["outd"]  # [D, NCH, 128, KB, BL]
        v = v.transpose(0, 1, 3, 2, 4).reshape(D, PP, BL)  # s = c*CH + kb*128 + n
        for d in range(D):
            perm = prep["perms"][d]
            cols = 1 + d * P + perm  # output column for sorted position s
            out[core * BL : (core + 1) * BL, cols] = v[d, :P].T
    out[:, 1:] += bout[None, 1:]
    return out


def kernel(**inputs):
    D, P, CH, NCH = D_FULL, P_FULL, 256, 4
    prep = _prepare(inputs, D, P, CH, NCH)
    nc = _build(prep, D, P, CH, NCH)
    res = _run_cores(nc, prep, np.asarray(inputs["embedding"], np.float32), NCORES)
    return _assemble(res.results, prep, inputs, D, P, CH, NCH, NCORES)
